# revision 1
# baseline (speedup 1.0000x reference)
"""Trainium2 Bass kernel for per-head-projection MultiHeadAttention.

Contract: kernel(**inputs) takes the FULL unsharded inputs (as produced by
reference.setup_inputs()) and returns the FULL [B, S, D] output.

Sharding (tensor-parallel over heads x data-parallel over batch):
  - 8 cores; cores 0-3 handle batch 0, cores 4-7 handle batch 1.
  - Each core owns 4 heads (two "head pairs"). It computes Q/K/V projections
    for those heads, causal attention, and a partial output projection
    (ctx @ Wo rows for its heads). The host sums the 4 partials per batch
    (the output linear is linear over head blocks) and adds bo.

Device structure (v2, pipelined over 512-row s-groups g=0..3):
  proj(g) -> V-transpose(g) -> attention(q-group g, both head pairs
  interleaved) -> normalize(g) -> output-projection(g). Attention consumes
  only K/V rows <= (g+1)*512 (causal), so everything streams.

Layouts: inputs pre-transposed on host (x^T [D, S]); projections emit
Q^T/K^T/V^T [128(=head pair), S]; scores are computed transposed
([keys, queries]) so softmax needs no transposes; the denominator comes
from a ones-column in V; exp runs fused over both heads ([128,1024] PSUM)
with the 1/sqrt(DH) scale folded in. All matmuls are float32r (full-rate
fp32). Diagonal causal tiles trim both the QK and PV matmul widths.
"""

import os
import sys

sys.path.insert(0, "/opt/trn_rl_repo")

import numpy as np

B, S, D, H = 2, 2048, 1024, 16
DH = D // H            # 64
NCORES = 8
HPC = H * B // NCORES  # 4 heads per core
NPAIR = HPC // 2       # 2 head pairs per core
SG = 512               # s-group / query-group size
NSG = S // SG          # 4
NKT = S // 128         # 16 key tiles
NDC = D // 128         # 8 contraction chunks

_BUILD_CACHE = {}


def _build(causal: bool):
    """Build + compile the per-core Bass program. Cached per causal flag."""
    import concourse.bass as bass
    import concourse.bacc as bacc
    import concourse.tile as tile
    from concourse import mybir

    f32 = mybir.dt.float32
    f32r = mybir.dt.float32r
    EXP = mybir.ActivationFunctionType.Exp

    nc = bacc.Bacc("TRN2", target_bir_lowering=False, debug=False)

    xq = nc.dram_tensor("xq", [D, S], f32r, kind="ExternalInput").ap()
    xk = nc.dram_tensor("xk", [D, S], f32r, kind="ExternalInput").ap()
    xv = nc.dram_tensor("xv", [D, S], f32r, kind="ExternalInput").ap()
    wq = nc.dram_tensor("wq", [NPAIR, D, 128], f32r, kind="ExternalInput").ap()
    wk = nc.dram_tensor("wk", [NPAIR, D, 128], f32r, kind="ExternalInput").ap()
    wv = nc.dram_tensor("wv", [NPAIR, D, 128], f32r, kind="ExternalInput").ap()
    wo = nc.dram_tensor("wo", [NPAIR, 128, D], f32r, kind="ExternalInput").ap()
    mk = nc.dram_tensor("mk", [128, 512], f32r, kind="ExternalInput").ap()
    on = nc.dram_tensor("on", [128, 64], f32r, kind="ExternalInput").ap()
    idm = nc.dram_tensor("idm", [128, 64], f32r, kind="ExternalInput").ap()
    bq = nc.dram_tensor("bq", [NPAIR, 128, 1], f32, kind="ExternalInput").ap()
    bk = nc.dram_tensor("bk", [NPAIR, 128, 1], f32, kind="ExternalInput").ap()
    bv = nc.dram_tensor("bv", [NPAIR, 128, 1], f32, kind="ExternalInput").ap()
    out = nc.dram_tensor("out", [S, D], f32, kind="ExternalOutput").ap()
    # DRAM bounce for the denominator inverses: DMA from DRAM supports
    # partition-broadcast (step-0) APs; SBUF sources and gpsimd
    # partition_broadcast (which always reads partition 0 on HW) do not.
    dscr = nc.dram_tensor("dscr", [NSG, HPC, SG], f32).ap()

    with tile.TileContext(nc) as tc:
        with (
            tc.tile_pool(name="persist", bufs=1) as persist,
            tc.tile_pool(name="xs", bufs=8) as xs_pool,
            tc.tile_pool(name="pts", bufs=4) as pt_pool,
            tc.tile_pool(name="vts", bufs=2) as vt_pool,
            tc.tile_pool(name="cxu", bufs=2) as cxu_pool,
            tc.tile_pool(name="outs", bufs=3) as out_pool,
            tc.tile_pool(name="smalls", bufs=2) as st_pool,
            tc.tile_pool(name="psma", bufs=2, space="PSUM") as psA,
            tc.tile_pool(name="psmb", bufs=2, space="PSUM") as psB,
            tc.tile_pool(name="psmc", bufs=2, space="PSUM") as psC,
        ):
            # consts (host-provided: identity blocks, diag mask). Weight and
            # const loads go through the gpsimd (SWDGE) queue so they don't
            # serialize ahead of the sync-queue activation streams; weight
            # tiles are split per (tensor, pair) for fine-grained deps.
            ident = persist.tile([128, 64], f32r, tag="ident")
            nc.gpsimd.dma_start(out=ident, in_=idm)
            mask = persist.tile([128, 512], f32r, tag="mask")
            nc.gpsimd.dma_start(out=mask, in_=mk)

            b_sb = persist.tile([128, 3, NPAIR], f32, tag="b")
            for t_i, bd in enumerate([bq, bk, bv]):
                for p in range(NPAIR):
                    nc.gpsimd.dma_start(out=b_sb[:, t_i, p : p + 1], in_=bd[p])

            w_sb = {}
            for t_i, wd in enumerate([wq, wk, wv]):
                for p in range(NPAIR):
                    wt = persist.tile(
                        [128, NDC, 128], f32r, tag=f"w{t_i}{p}", name=f"w{t_i}{p}"
                    )
                    w_sb[t_i, p] = wt
                    for c in range(NDC):
                        nc.gpsimd.dma_start(
                            out=wt[:, c, :], in_=wd[p, c * 128 : (c + 1) * 128, :]
                        )
            wo_sb = persist.tile([128, NPAIR, D], f32r, tag="wo")
            for p in range(NPAIR):
                nc.gpsimd.dma_start(out=wo_sb[:, p, :], in_=wo[p])

            qT = persist.tile([128, NPAIR, S], f32r, tag="qT")
            kT = persist.tile([128, NPAIR, S], f32r, tag="kT")
            vN = persist.tile([128, HPC, NKT, 65], f32r, tag="vN")
            ctxn = persist.tile([128, NPAIR, S], f32r, tag="ctxn")

            # ones column of V-natural (softmax denominator trick)
            nc.sync.dma_start(
                out=vN[:, :, :, 64], in_=on.rearrange("p (h k) -> p h k", h=HPC)
            )

            # Causal: fully pipelined proj(g)->attn(g). Non-causal:
            # attention needs ALL key tiles, so run every projection phase
            # first (avoids attention holding PSUM slots while waiting on
            # later-g projections).
            if causal:
                g_phases = [(g, True, True) for g in range(NSG)]
            else:
                g_phases = [(g, True, False) for g in range(NSG)] + [
                    (g, False, True) for g in range(NSG)
                ]
            for g, do_proj, do_attn in g_phases:
                gs = slice(g * SG, (g + 1) * SG)

                if do_proj:
                    # ---- projections for s-group g (both pairs share a 2-bank
                    # PSUM tile: pair p in columns [p*SG:(p+1)*SG]) ----
                    vtg = vt_pool.tile([128, NPAIR, SG], f32r, tag="vtg", name="vtg")
                    for t_i, xd in enumerate([xq, xk, xv]):
                        pp = psA.tile([128, 2 * SG], f32, tag="sc2", name="pp")
                        for c in range(NDC):
                            xc = xs_pool.tile([128, SG], f32r, tag="xc", name="xc")
                            nc.sync.dma_start(
                                out=xc, in_=xd[c * 128 : (c + 1) * 128, gs]
                            )
                            for p in range(NPAIR):
                                nc.tensor.matmul(
                                    pp[:, p * SG : (p + 1) * SG],
                                    lhsT=w_sb[t_i, p][:, c, :],
                                    rhs=xc,
                                    start=(c == 0),
                                    stop=(c == NDC - 1),
                                )
                        for p in range(NPAIR):
                            dst = (
                                qT[:, p, gs]
                                if t_i == 0
                                else (kT[:, p, gs] if t_i == 1 else vtg[:, p, :])
                            )
                            nc.vector.tensor_scalar_add(
                                out=dst,
                                in0=pp[:, p * SG : (p + 1) * SG],
                                scalar1=b_sb[:, t_i, p : p + 1],
                            )

                    # ---- V -> natural layout for this group's 4 key tiles ----
                    for p in range(NPAIR):
                        for h_s in range(2):
                            h = 2 * p + h_s
                            hp = slice(h_s * 64, (h_s + 1) * 64)
                            for k4 in range(4):
                                tp_ps = psC.tile([128, 64], f32r, tag="mm", name="tp")
                                nc.tensor.transpose(
                                    tp_ps,
                                    in_=vtg[hp, p, k4 * 128 : (k4 + 1) * 128],
                                    identity=ident[hp, :],
                                )
                                nc.vector.tensor_copy(
                                    out=vN[:, h, 4 * g + k4, 0:64], in_=tp_ps
                                )

                if do_attn:
                    # ---- attention for q-group g ----
                    ctxu = cxu_pool.tile([128, NPAIR, SG], f32, tag="ctxu", name="ctxu")
                    nkc = (4 * g + 4) if causal else NKT
                    for p in range(NPAIR):
                        ctx2 = [
                            psB.tile([65, SG], f32, tag="ctx", name="ctx")
                            for _ in range(2)
                        ]

                        def emit_pv(kc, tp_i, pt2, ctx2=ctx2, p=p, nkc=nkc):
                            pvoff = tp_i * 128 if (causal and tp_i > 0) else 0
                            for h_s in range(2):
                                nc.tensor.matmul(
                                    ctx2[h_s][:, pvoff:SG],
                                    lhsT=vN[:, 2 * p + h_s, kc, :],
                                    rhs=pt2[:, h_s * SG + pvoff : (h_s + 1) * SG],
                                    start=(kc == 0),
                                    stop=(kc == nkc - 1),
                                )

                        # software-pipelined: sc/exp(kc) emitted before pv(kc-1)
                        prev = None
                        for kc in range(nkc):
                            tp_i = kc - 4 * g
                            diag = causal and tp_i >= 0
                            off = min(tp_i * 128, 256) if diag else 0
                            sc2 = psA.tile([128, 2 * SG], f32, tag="sc2", name="sc2")
                            for h_s in range(2):
                                hp = slice(h_s * 64, (h_s + 1) * 64)
                                nc.tensor.matmul(
                                    sc2[:, h_s * SG + off : (h_s + 1) * SG],
                                    lhsT=kT[hp, p, kc * 128 : (kc + 1) * 128],
                                    rhs=qT[hp, p, g * SG + off : (g + 1) * SG],
                                    start=True,
                                    stop=True,
                                )
                            pt2 = pt_pool.tile([128, 2 * SG], f32r, tag="pt", name="pt2")
                            if off == 0:
                                nc.scalar.activation(pt2, sc2, EXP, scale=0.125)
                            else:
                                for h_s in range(2):
                                    nc.scalar.activation(
                                        pt2[:, h_s * SG + off : (h_s + 1) * SG],
                                        sc2[:, h_s * SG + off : (h_s + 1) * SG],
                                        EXP,
                                        scale=0.125,
                                    )
                            if diag:
                                d0 = tp_i * 128
                                for h_s in range(2):
                                    nc.vector.tensor_mul(
                                        pt2[:, h_s * SG + d0 : h_s * SG + d0 + 128],
                                        pt2[:, h_s * SG + d0 : h_s * SG + d0 + 128],
                                        mask[:, 384:512],
                                    )
                            if prev is not None:
                                emit_pv(*prev)
                            prev = (kc, tp_i, pt2)
                        emit_pv(*prev)

                        # ---- per-pair normalize (overlaps the other pair /
                        # next phase): stash denominators at 32-aligned rows,
                        # one [64,512] reciprocal, DRAM-bounce broadcast, one
                        # full-width multiply ----
                        stage = st_pool.tile([64, SG], f32, tag="stage", name="stage")
                        nc.vector.memset(stage, 1.0)
                        for h_s in range(2):
                            hp = slice(h_s * 64, (h_s + 1) * 64)
                            nc.vector.tensor_copy(
                                stage[32 * h_s : 32 * h_s + 1, :], ctx2[h_s][64:65, :]
                            )
                            nc.vector.tensor_copy(ctxu[hp, p, :], ctx2[h_s][0:64, :])
                        inv = st_pool.tile([64, SG], f32, tag="inv", name="inv")
                        nc.vector.reciprocal(inv, stage)
                        for h_s in range(2):
                            nc.sync.dma_start(
                                out=dscr[g, 2 * p + h_s],
                                in_=inv[32 * h_s : 32 * h_s + 1, :],
                            )
                        rb = st_pool.tile([128, SG], f32, tag="rb", name="rb")
                        for h_s in range(2):
                            nc.sync.dma_start(
                                out=rb[h_s * 64 : (h_s + 1) * 64, :],
                                in_=dscr[g, 2 * p + h_s].partition_broadcast(64),
                            )
                        nc.vector.tensor_mul(ctxn[:, p, gs], ctxu[:, p, :], rb)

                    # ---- partial output projection for s-group g ----
                    for st4 in range(4):
                        srow = (4 * g + st4) * 128
                        for n in range(D // SG):
                            op = psC.tile([128, SG], f32, tag="mm", name="op")
                            for p in range(NPAIR):
                                nc.tensor.matmul(
                                    op,
                                    lhsT=ctxn[:, p, srow : srow + 128],
                                    rhs=wo_sb[:, p, n * SG : (n + 1) * SG],
                                    start=(p == 0),
                                    stop=(p == NPAIR - 1),
                                )
                            ob = out_pool.tile([128, SG], f32, tag="ob", name="ob")
                            nc.vector.tensor_copy(ob, op)
                            nc.sync.dma_start(
                                out=out[srow : srow + 128, n * SG : (n + 1) * SG],
                                in_=ob,
                            )


    nc.compile()
    return nc


def _core_inputs(query, key, value, Wq, bq, Wk, bk, Wv, bv, Wo, core):
    b = core // (NCORES // B)
    h0 = (core % (NCORES // B)) * HPC
    f32 = np.float32

    def packw(W):
        # [H, D, DH] -> per-pair [D, 128] stacks
        return np.ascontiguousarray(
            np.stack(
                [
                    np.concatenate([W[h0 + 2 * p], W[h0 + 2 * p + 1]], axis=1)
                    for p in range(NPAIR)
                ]
            ),
            dtype=f32,
        )

    def packb(bias):
        return np.ascontiguousarray(
            np.stack(
                [
                    np.concatenate([bias[h0 + 2 * p], bias[h0 + 2 * p + 1]])
                    for p in range(NPAIR)
                ]
            ).reshape(NPAIR, 128, 1),
            dtype=f32,
        )

    wo_p = np.ascontiguousarray(
        np.stack(
            [Wo[(h0 + 2 * p) * DH : (h0 + 2 * p + 2) * DH] for p in range(NPAIR)]
        ),
        dtype=f32,
    )
    jj, ii = np.meshgrid(np.arange(128), np.arange(128), indexing="ij")
    mkk = np.zeros((128, 512), f32)
    mkk[:, 384:512] = (jj <= ii).astype(f32)
    return {
        "mk": mkk,
        "on": np.ones((128, 64), f32),
        "idm": np.concatenate([np.eye(64, dtype=f32)] * 2, axis=0),
        "xq": np.ascontiguousarray(query[b].T, dtype=f32),
        "xk": np.ascontiguousarray(key[b].T, dtype=f32),
        "xv": np.ascontiguousarray(value[b].T, dtype=f32),
        "wq": packw(Wq),
        "wk": packw(Wk),
        "wv": packw(Wv),
        "wo": wo_p,
        "bq": packb(bq),
        "bk": packb(bk),
        "bv": packb(bv),
    }


LAST_RESULTS = None


def kernel(query, key, value, Wq, bq, Wk, bk, Wv, bv, Wo, bo, look_ahead_mask):
    global LAST_RESULTS
    from concourse.bass_utils import run_bass_kernel_spmd

    query = np.asarray(query, dtype=np.float32)
    key = np.asarray(key, dtype=np.float32)
    value = np.asarray(value, dtype=np.float32)
    Wq, Wk, Wv = (np.asarray(a, dtype=np.float32) for a in (Wq, Wk, Wv))
    bq, bk, bv = (np.asarray(a, dtype=np.float32) for a in (bq, bk, bv))
    Wo = np.asarray(Wo, dtype=np.float32)
    bo = np.asarray(bo, dtype=np.float32)
    causal = bool(np.asarray(look_ahead_mask).item())

    if causal not in _BUILD_CACHE:
        _BUILD_CACHE[causal] = _build(causal)
    nc = _BUILD_CACHE[causal]

    in_maps = [
        _core_inputs(query, key, value, Wq, bq, Wk, bk, Wv, bv, Wo, c)
        for c in range(NCORES)
    ]
    res = run_bass_kernel_spmd(nc, in_maps, core_ids=list(range(NCORES)))
    LAST_RESULTS = res

    gpb = NCORES // B
    out = np.stack(
        [
            np.sum([res.results[b * gpb + i]["out"] for i in range(gpb)], axis=0)
            for b in range(B)
        ]
    )
    return (out + bo[None, None, :]).astype(np.float32)



# revision 9
# speedup vs baseline: 1.4702x; 1.4702x over previous
"""Trainium2 Bass kernel for per-head-projection MultiHeadAttention.

Contract: kernel(**inputs) takes the FULL unsharded inputs (as produced by
reference.setup_inputs()) and returns the FULL [B, S, D] output.

Sharding (tensor-parallel over heads x data-parallel over batch):
  - 8 cores; cores 0-3 handle batch 0, cores 4-7 handle batch 1.
  - Each core owns 4 heads (two "head pairs"). It computes Q/K/V projections
    for those heads, causal attention, and a partial output projection
    (ctx @ Wo rows for its heads). The host sums the 4 partials per batch
    (bf16) and adds bo.

v4 design notes (engines are in-order; overlap = emission interleaving):
  - All matmul operands bf16 (fp32 matmuls are 2-pass half-rate on TRN2).
    PSUM accumulation fp32; output partials bf16.
  - Scores are computed transposed ([keys, queries]) with both heads of a
    pair row-packed into the PE array (K=64 tiles at partition 0/64), so
    softmax needs no transposes; the denominator comes from a ones-column
    in V-natural; exp runs fused over both heads as one strided
    [128,2,W] ACTIVATE with the 1/sqrt(DH) scale folded in.
  - V is projected directly into natural [keys, dims] layout with the
    activations as the stationary operand and BOTH pairs' Wv moving
    (N=256), killing the v2 PE transposes.
  - sc2 PSUM is triple-buffered (6 banks) so the scores matmul of kc+1
    never serializes behind exp(kc-1); ctx accumulators take the other 2.
  - The attention inner loop is ACT(exp)-bound; PE-only work (V-nat for
    g+1, output projection for g-1, deferred softmax-normalize rank-1
    broadcasts) is interleaved into it as "fillers".
  - Normalize: ones-column sums -> partition-aligned copy -> one
    reciprocal_approx_fast -> rank-1 ones-block matmul broadcast (PSUM) ->
    SBUF copy -> per-head multiply into bf16 ctxn. The PE parts are
    deferred past the next Q/K projection so the reciprocal latency never
    stalls the in-order PE queue.
  - Startup: g=0 loads run on three queues in parallel (xq sync, xv
    scalar, weights gpsimd); output stores alternate sync/gpsimd.
"""

import os
import sys

sys.path.insert(0, "/opt/trn_rl_repo")

import numpy as np

B, S, D, H = 2, 2048, 1024, 16
DH = D // H            # 64
NCORES = 8
HPC = H * B // NCORES  # 4 heads per core
NPAIR = HPC // 2       # 2 head pairs per core
SG = 512               # s-group / query-group size
NSG = S // SG          # 4
NKT = S // 128         # 16 key tiles
NDC = D // 128         # 8 contraction chunks

_BUILD_CACHE = {}


def _build(causal: bool, bz: bool, debug_dumps: bool = False):
    """Build + compile the per-core Bass program.

    bz: all of bq/bk/bv are zero -> skip bias adds (plain casts instead).
    """
    import concourse.bass as bass
    import concourse.bacc as bacc
    import concourse.tile as tile
    from concourse import mybir

    f32 = mybir.dt.float32
    bf16 = mybir.dt.bfloat16
    EXP = mybir.ActivationFunctionType.Exp

    nc = bacc.Bacc("TRN2", target_bir_lowering=False, debug=False)

    xq = nc.dram_tensor("xq", [D, S], bf16, kind="ExternalInput").ap()
    xk = nc.dram_tensor("xk", [D, S], bf16, kind="ExternalInput").ap()
    xv = nc.dram_tensor("xv", [D, S], bf16, kind="ExternalInput").ap()
    wq = nc.dram_tensor("wq", [NPAIR, D, 128], bf16, kind="ExternalInput").ap()
    wk = nc.dram_tensor("wk", [NPAIR, D, 128], bf16, kind="ExternalInput").ap()
    wv = nc.dram_tensor("wv", [D, NPAIR * 128], bf16, kind="ExternalInput").ap()
    wo = nc.dram_tensor("wo", [NPAIR, 128, D], bf16, kind="ExternalInput").ap()
    mk = nc.dram_tensor("mk", [128, 128], bf16, kind="ExternalInput").ap()
    on = nc.dram_tensor("on", [128, 64], bf16, kind="ExternalInput").ap()
    bqk = nc.dram_tensor("bqk", [2, NPAIR, 128, 1], f32, kind="ExternalInput").ap()
    bvr = nc.dram_tensor("bvr", [1, NPAIR * 128], bf16, kind="ExternalInput").ap()
    on1 = nc.dram_tensor("on1", [1, 128], bf16, kind="ExternalInput").ap()
    obk = nc.dram_tensor("obk", [33, 128], f32, kind="ExternalInput").ap()
    out = nc.dram_tensor("out", [S, D], bf16, kind="ExternalOutput").ap()
    if debug_dumps:
        d_qT = nc.dram_tensor("d_qT", [128, NPAIR, S], bf16, kind="ExternalOutput").ap()
        d_kT = nc.dram_tensor("d_kT", [128, NPAIR, S], bf16, kind="ExternalOutput").ap()
        d_vN = nc.dram_tensor(
            "d_vN", [128, NPAIR, NKT, 2, 65], bf16, kind="ExternalOutput"
        ).ap()
        d_ctxn = nc.dram_tensor(
            "d_ctxn", [128, NPAIR, S], bf16, kind="ExternalOutput"
        ).ap()

    with tile.TileContext(nc) as tc:
        with (
            tc.tile_pool(name="persist", bufs=1) as persist,
            tc.tile_pool(name="xs", bufs=6) as xs_pool,
            tc.tile_pool(name="pts", bufs=4) as pt_pool,
            tc.tile_pool(name="outs", bufs=6) as out_pool,
            tc.tile_pool(name="rbs", bufs=2) as rb_pool,
            # psA (3 x 2 banks) serves pp/sc2/vnat/rb/op; psB (2 x 1 bank)
            # holds one pair's ctx accumulators at a time.
            tc.tile_pool(name="psma", bufs=3, space="PSUM") as psA,
            tc.tile_pool(name="psmb", bufs=2, space="PSUM") as psB,
        ):
            # ---- weights first (first consumer), then consts; all via the
            # gpsimd (SWDGE) queue so they don't block the sync-queue x loads.
            w_sb = {}
            for t_i, wd in enumerate([wq, wk]):
                for p in range(NPAIR):
                    wt = persist.tile(
                        [128, NDC, 128], bf16, tag=f"w{t_i}{p}", name=f"w{t_i}{p}"
                    )
                    w_sb[t_i, p] = wt
                    nc.gpsimd.dma_start(
                        out=wt, in_=wd[p].rearrange("(c q) e -> q c e", q=128)
                    )
            wv_sb = persist.tile([128, NDC, 256], bf16, tag="wv")
            nc.gpsimd.dma_start(
                out=wv_sb, in_=wv.rearrange("(c q) e -> q c e", q=128)
            )
            wo_sb = persist.tile([128, NPAIR, D], bf16, tag="wo")
            for p in range(NPAIR):
                nc.gpsimd.dma_start(out=wo_sb[:, p, :], in_=wo[p])

            mask = persist.tile([128, 128], bf16, tag="mask")
            nc.gpsimd.dma_start(out=mask, in_=mk)
            obk_sb = persist.tile([33, 128], f32, tag="obk")
            nc.gpsimd.dma_start(out=obk_sb, in_=obk)
            if not bz:
                b_sb = persist.tile([128, 2, NPAIR, 1], f32, tag="b")
                nc.gpsimd.dma_start(out=b_sb, in_=bqk.rearrange("t p q o -> q t p o"))
                bv_sb = persist.tile([1, 256], bf16, tag="bv")
                nc.gpsimd.dma_start(out=bv_sb, in_=bvr)
                on1_sb = persist.tile([1, 128], bf16, tag="on1")
                nc.gpsimd.dma_start(out=on1_sb, in_=on1)

            qT = persist.tile([128, NPAIR, S], bf16, tag="qT")
            kT = persist.tile([128, NPAIR, S], bf16, tag="kT")
            # V natural layout: per (pair, key-tile) two 65-wide head blocks
            # (64 dims + denominator ones column).
            vN = persist.tile([128, NPAIR, NKT, 2, 65], bf16, tag="vN")
            ctxn = persist.tile([128, NPAIR, S], bf16, tag="ctxn")

            # ones columns of V-natural (softmax denominator trick)
            nc.gpsimd.dma_start(
                out=vN[:, :, :, :, 64],
                in_=on.rearrange("q (p k h) -> q p k h", p=NPAIR, k=NKT),
            )

            # ping-pong fp32 staging rows for the denominators (rows 0 and 32
            # hold the two heads' values; other rows are memset once so the
            # reciprocal and the rank-1 broadcast matmul never see garbage).
            stages = []
            for i in range(2):
                stg = persist.tile([33, SG], f32, tag=f"stg{i}", name=f"stg{i}")
                stgr = persist.tile([33, SG], f32, tag=f"stgr{i}", name=f"stgr{i}")
                nc.vector.memset(stg, 1.0)
                stages.append((stg, stgr))

            x_sb = {}       # (t_i, g) -> SBUF tile [128, NDC, SG]
            out_q = [0]     # round-robin selector for output stores

            def emit_xload(g, engines=None):
                for t_i, xd in enumerate([xq, xk, xv]):
                    eng = (engines or [nc.sync] * 3)[t_i]
                    xt = xs_pool.tile(
                        [128, NDC, SG], bf16, tag="xt", name=f"x{t_i}g{g}"
                    )
                    xr = xd.rearrange("(c q) s -> q c s", q=128)
                    gs = slice(g * SG, (g + 1) * SG)
                    eng.dma_start(out=xt[:, 0:4, :], in_=xr[:, 0:4, gs])
                    eng.dma_start(out=xt[:, 4:8, :], in_=xr[:, 4:8, gs])
                    x_sb[t_i, g] = xt

            def emit_qk_proj(t_i, g):
                # q/k projections for s-group g -> qT/kT (transposed layout)
                gs = slice(g * SG, (g + 1) * SG)
                pp = psA.tile([128, 2, SG], f32, tag="big", name="pp")
                for c in range(NDC):
                    for p in range(NPAIR):
                        nc.tensor.matmul(
                            pp[:, p, :],
                            lhsT=w_sb[t_i, p][:, c, :],
                            rhs=x_sb[t_i, g][:, c, :],
                            start=(c == 0),
                            stop=(c == NDC - 1),
                        )
                dstT = qT if t_i == 0 else kT
                for p in range(NPAIR):
                    if bz:
                        nc.vector.tensor_copy(dstT[:, p, gs], pp[:, p, :])
                    else:
                        nc.vector.tensor_scalar_add(
                            out=dstT[:, p, gs],
                            in0=pp[:, p, :],
                            scalar1=b_sb[:, t_i, p, :],
                        )

            def emit_vnat_unit(kt):
                # V-natural projection for one key-tile, both pairs at once:
                # activations stationary, both pairs' Wv moving (N=256).
                g = kt // 4
                k4 = kt % 4
                vp = psA.tile([128, 2, 2, 64], f32, tag="big", name="vp")
                for c in range(NDC):
                    nc.tensor.matmul(
                        vp,
                        lhsT=x_sb[2, g][:, c, k4 * 128 : (k4 + 1) * 128],
                        rhs=wv_sb[:, c, :],
                        start=(c == 0),
                        stop=(bz and c == NDC - 1),
                    )
                if not bz:
                    nc.tensor.matmul(
                        vp, lhsT=on1_sb, rhs=bv_sb, start=False, stop=True
                    )
                for p in range(NPAIR):
                    nc.vector.tensor_copy(
                        out=vN[:, p, kt, :, 0:64], in_=vp[:, p, :, :]
                    )

            def emit_outproj_unit(g, st4, n):
                # one [128s x 512n] tile of the partial output projection
                srow = (4 * g + st4) * 128
                op = psA.tile([128, SG], f32, tag="big", name="op")
                for p in range(NPAIR):
                    nc.tensor.matmul(
                        op,
                        lhsT=ctxn[:, p, srow : srow + 128],
                        rhs=wo_sb[:, p, n * SG : (n + 1) * SG],
                        start=(p == 0),
                        stop=(p == NPAIR - 1),
                    )
                ob = out_pool.tile([128, SG], bf16, tag="ob", name="ob")
                nc.vector.tensor_copy(ob, op)
                eng = nc.sync if out_q[0] % 2 == 0 else nc.gpsimd
                out_q[0] += 1
                eng.dma_start(
                    out=out[srow : srow + 128, n * SG : (n + 1) * SG], in_=ob
                )

            def emit_attention(g, fillers):
                """Attention for q-group g. Pops one filler closure after each
                score group. Returns the deferred normalize-finish closure of
                the last pair (caller emits it after the next Q/K proj)."""
                gs = slice(g * SG, (g + 1) * SG)
                nkc = (4 * g + 4) if causal else NKT
                fin = None
                for p in range(NPAIR):
                    ctx2 = [
                        psB.tile([65, SG], f32, tag="ctx", name="ctx")
                        for _ in range(2)
                    ]

                    def emit_pv(kc, off, pt2, ctx2=ctx2, p=p, nkc=nkc):
                        for h_s in range(2):
                            nc.tensor.matmul(
                                ctx2[h_s][:, off:SG],
                                lhsT=vN[:, p, kc, h_s, :],
                                rhs=pt2[:, h_s, off:],
                                start=(kc == 0),
                                stop=(kc == nkc - 1),
                            )

                    # software-pipelined: sc/exp(kc) emitted before pv(kc-1)
                    prev = None
                    for kc in range(nkc):
                        tp_i = kc - 4 * g
                        diag = causal and tp_i >= 0
                        off = tp_i * 128 if diag else 0
                        sc2 = psA.tile([128, 2, SG], f32, tag="big", name="sc2")
                        for h_s in range(2):
                            hp = slice(h_s * 64, (h_s + 1) * 64)
                            nc.tensor.matmul(
                                sc2[:, h_s, off:],
                                lhsT=kT[hp, p, kc * 128 : (kc + 1) * 128],
                                rhs=qT[hp, p, g * SG + off : (g + 1) * SG],
                                start=True,
                                stop=True,
                            )
                        pt2 = pt_pool.tile([128, 2, SG], bf16, tag="pt", name="pt2")
                        nc.scalar.activation(
                            pt2[:, :, off:], sc2[:, :, off:], EXP, scale=0.125
                        )
                        if diag:
                            d0 = tp_i * 128
                            for h_s in range(2):
                                nc.vector.tensor_mul(
                                    pt2[:, h_s, d0 : d0 + 128],
                                    pt2[:, h_s, d0 : d0 + 128],
                                    mask,
                                )
                        if fillers:
                            fillers.pop(0)()
                        if prev is not None:
                            emit_pv(*prev)
                        prev = (kc, off, pt2)
                    emit_pv(*prev)

                    # ---- normalize, phase 1 (DVE only): stage the ones-column
                    # sums at partition 0/32 and take fast reciprocals.
                    stg, stgr = stages[(2 * g + p) % 2]
                    for h_s in range(2):
                        nc.vector.tensor_copy(
                            stg[32 * h_s : 32 * h_s + 1, :], ctx2[h_s][64:65, :]
                        )
                    nc.vector.reciprocal_approx_fast(out=stgr, in_=stg)

                    def fin_fn(p=p, stgr=stgr, ctx2=ctx2):
                        # phase 2: rank-1 broadcast (PE) + per-head multiply
                        rb = psA.tile([128, SG], f32, tag="big", name="rb")
                        nc.tensor.matmul(
                            rb, lhsT=obk_sb, rhs=stgr, start=True, stop=True
                        )
                        rbs = rb_pool.tile([128, SG], f32, tag="rbs", name="rbs")
                        nc.vector.tensor_copy(rbs, rb)
                        for h_s in range(2):
                            hp = slice(h_s * 64, (h_s + 1) * 64)
                            nc.vector.tensor_mul(
                                ctxn[hp, p, gs], ctx2[h_s][0:64, :], rbs[hp, :]
                            )

                    if p == 0:
                        # inject pair0's finish into pair1's filler stream
                        fillers.insert(min(1, len(fillers)), fin_fn)
                    else:
                        fin = fin_fn
                # drain leftover fillers
                while fillers:
                    fillers.pop(0)()
                return fin

            if causal:
                emit_xload(0, engines=[nc.sync, nc.sync, nc.scalar])
                emit_qk_proj(0, 0)
                emit_qk_proj(1, 0)
                for kt in range(4):
                    emit_vnat_unit(kt)
                for g in range(NSG):
                    if g + 1 < NSG:
                        emit_xload(g + 1)
                    fillers = []
                    if g > 0:
                        for st4 in range(4):
                            for n in range(D // SG):
                                fillers.append(
                                    lambda g=g, st4=st4, n=n: emit_outproj_unit(
                                        g - 1, st4, n
                                    )
                                )
                    if g + 1 < NSG:
                        vts = [
                            (lambda kt=kt: emit_vnat_unit(kt))
                            for kt in range(4 * g + 4, 4 * g + 8)
                        ]
                        if fillers:
                            # interleave vnat units among the outproj units
                            mixed = []
                            for i, f in enumerate(fillers):
                                mixed.append(f)
                                if i % 2 == 1 and vts:
                                    mixed.append(vts.pop(0))
                            fillers = mixed + vts
                        else:
                            fillers = vts
                    fin = emit_attention(g, fillers)
                    if g + 1 < NSG:
                        emit_qk_proj(0, g + 1)
                        emit_qk_proj(1, g + 1)
                    fin()
                for st4 in range(4):
                    for n in range(D // SG):
                        emit_outproj_unit(NSG - 1, st4, n)
                if debug_dumps:
                    nc.sync.dma_start(out=d_qT, in_=qT[:, :, :])
                    nc.sync.dma_start(out=d_kT, in_=kT[:, :, :])
                    nc.sync.dma_start(out=d_vN, in_=vN[:, :, :, :, :])
                    nc.sync.dma_start(out=d_ctxn, in_=ctxn[:, :, :])
            else:
                # non-causal: attention needs ALL key tiles -> run every
                # projection first, then attention with outproj fillers.
                emit_xload(0, engines=[nc.sync, nc.sync, nc.scalar])
                for g in range(1, NSG):
                    emit_xload(g)
                for g in range(NSG):
                    emit_qk_proj(0, g)
                    emit_qk_proj(1, g)
                    for kt in range(4 * g, 4 * g + 4):
                        emit_vnat_unit(kt)
                for g in range(NSG):
                    fillers = []
                    if g > 0:
                        for st4 in range(4):
                            for n in range(D // SG):
                                fillers.append(
                                    lambda g=g, st4=st4, n=n: emit_outproj_unit(
                                        g - 1, st4, n
                                    )
                                )
                    fin = emit_attention(g, fillers)
                    fin()
                for st4 in range(4):
                    for n in range(D // SG):
                        emit_outproj_unit(NSG - 1, st4, n)

    nc.compile()
    return nc


def _core_inputs(query, key, value, Wq, bq, Wk, bk, Wv, bv, Wo, core):
    import ml_dtypes

    bf16 = ml_dtypes.bfloat16
    b = core // (NCORES // B)
    h0 = (core % (NCORES // B)) * HPC
    f32 = np.float32

    def packw(W):
        # [H, D, DH] -> per-pair [D, 128] stacks
        return np.ascontiguousarray(
            np.stack(
                [
                    np.concatenate([W[h0 + 2 * p], W[h0 + 2 * p + 1]], axis=1)
                    for p in range(NPAIR)
                ]
            ).astype(bf16)
        )

    def packb(bias):
        return np.ascontiguousarray(
            np.stack(
                [
                    np.concatenate([bias[h0 + 2 * p], bias[h0 + 2 * p + 1]])
                    for p in range(NPAIR)
                ]
            ).reshape(NPAIR, 128, 1),
            dtype=f32,
        )

    wo_p = np.ascontiguousarray(
        np.stack(
            [Wo[(h0 + 2 * p) * DH : (h0 + 2 * p + 2) * DH] for p in range(NPAIR)]
        ).astype(bf16)
    )
    wv_p = np.ascontiguousarray(
        np.concatenate(
            [
                np.concatenate([Wv[h0 + 2 * p], Wv[h0 + 2 * p + 1]], axis=1)
                for p in range(NPAIR)
            ],
            axis=1,
        ).astype(bf16)
    )  # [D, 256]
    kk, qq = np.meshgrid(np.arange(128), np.arange(128), indexing="ij")
    mkk = (kk <= qq).astype(bf16)  # key <= query (keys on partitions)
    obk = np.zeros((33, 128), f32)
    obk[0, 0:64] = 1.0
    obk[32, 64:128] = 1.0
    bvr = np.concatenate(
        [
            np.concatenate([bv[h0 + 2 * p], bv[h0 + 2 * p + 1]])
            for p in range(NPAIR)
        ]
    ).reshape(1, 256).astype(bf16)
    return {
        "mk": mkk,
        "on": np.ones((128, 64), bf16),
        "on1": np.ones((1, 128), bf16),
        "obk": obk,
        "xq": np.ascontiguousarray(query[b].T.astype(bf16)),
        "xk": np.ascontiguousarray(key[b].T.astype(bf16)),
        "xv": np.ascontiguousarray(value[b].T.astype(bf16)),
        "wq": packw(Wq),
        "wk": packw(Wk),
        "wv": wv_p,
        "wo": wo_p,
        "bqk": np.stack([packb(bq), packb(bk)]),
        "bvr": bvr,
    }


LAST_RESULTS = None


def kernel(query, key, value, Wq, bq, Wk, bk, Wv, bv, Wo, bo, look_ahead_mask):
    global LAST_RESULTS
    from concourse.bass_utils import run_bass_kernel_spmd

    query = np.asarray(query, dtype=np.float32)
    key = np.asarray(key, dtype=np.float32)
    value = np.asarray(value, dtype=np.float32)
    Wq, Wk, Wv = (np.asarray(a, dtype=np.float32) for a in (Wq, Wk, Wv))
    bq, bk, bv = (np.asarray(a, dtype=np.float32) for a in (bq, bk, bv))
    Wo = np.asarray(Wo, dtype=np.float32)
    bo = np.asarray(bo, dtype=np.float32)
    causal = bool(np.asarray(look_ahead_mask).item())
    bz = not (np.any(bq) or np.any(bk) or np.any(bv))

    if (causal, bz) not in _BUILD_CACHE:
        _BUILD_CACHE[causal, bz] = _build(causal, bz)
    nc = _BUILD_CACHE[causal, bz]

    in_maps = [
        _core_inputs(query, key, value, Wq, bq, Wk, bk, Wv, bv, Wo, c)
        for c in range(NCORES)
    ]
    res = run_bass_kernel_spmd(nc, in_maps, core_ids=list(range(NCORES)))
    LAST_RESULTS = res

    gpb = NCORES // B
    out = np.stack(
        [
            np.sum(
                [
                    res.results[b * gpb + i]["out"].astype(np.float32)
                    for i in range(gpb)
                ],
                axis=0,
            )
            for b in range(B)
        ]
    )
    return (out + bo[None, None, :]).astype(np.float32)
